# revision 22
# baseline (speedup 1.0000x reference)
"""Trainium2 Bass kernel for CrossAttentionFusion.

Math (kv seq_len == 1 collapses attention to two chained linear layers):
    eeg_att = ecg @ (Wo1 @ Wv1).T + (bv1 @ Wo1.T + bo1)
    eeg_out = LN(eeg + eeg_att) * g1 + beta1
    ecg_att = eeg @ (Wo2 @ Wv2).T + (bv2 @ Wo2.T + bo2)
    ecg_out = LN(ecg + ecg_att) * g2 + beta2
    out     = eeg_out @ WfL.T + ecg_out @ WfR.T + bf     (Wf = [WfL | WfR])

g/beta are folded into the fusion weights on the host:
    out = z1 @ (WfL*g1).T + z2 @ (WfR*g2).T + (bf + beta1@WfL.T + beta2@WfR.T)
where z = (a - mean(a)) * rsqrt(var(a) + eps) is the bare standardization.

The host supplies BOTH layouts of x (straight [rows, D] for the residual
path and transposed [D, rows] for the matmul lhsT), so the device does no
casting and no DMA/PE input transposes: the PE runs a dense back-to-back
matmul stream from the first microsecond.

The attention matmuls run in fp8-e4m3 with DoubleRow perf mode (2 fp8
MACs/cell/cycle): xT is quantized at scale 1, the folded attention weights
at scale 32. Because LN is scale-invariant, the 32x factor is folded into
the straight-x residual (host-scaled bf16) and eps (x1024) instead of being
divided out on-device. The fused matmul stays bf16 (fp8 there would breach
the error budget; measured 1.1e-2 vs 3.2e-2 rel err).

Per 128-row block:
  fp8 DoubleRow attention matmul (xT chunk stationary, W.T moving,
  rows-on-partition PSUM) -> residual + LN on DVE -> PE-transpose z ->
  bf16 fused matmul -> f32 store (psum->sbuf copy on the scalar engine).

Sharding: pure data parallel over the batch dim across 8 NeuronCores.
"""

import numpy as np
import ml_dtypes

import concourse.bass as bass
import concourse.mybir as mybir
import concourse.tile as tile
from concourse import bacc
from concourse.masks import make_identity

B, D = 32768, 1024
N_CORES = 8
ROWS_PER_CORE = B // N_CORES
EPS = 1e-5
W_SCALE = 32.0  # fp8 scale on attention weights; folded into residual + eps
F32 = mybir.dt.float32
BF16 = mybir.dt.bfloat16
FP8 = mybir.dt.float8e4
PM_DR = mybir.MatmulPerfMode.DoubleRow
BLK = 128  # row block (psum partition tile)
SB = 512  # super-block rows per strip
ts = bass.ts
AF = mybir.ActivationFunctionType
ALU = mybir.AluOpType


def build_program(n_rows=ROWS_PER_CORE, use_b1=False, use_b2=False, use_bf=False):
    nc = bacc.Bacc("TRN2", target_bir_lowering=False, debug=False)
    x1 = nc.dram_tensor("x1", (n_rows, D), BF16, kind="ExternalInput").ap()
    x2 = nc.dram_tensor("x2", (n_rows, D), BF16, kind="ExternalInput").ap()
    x1tr = nc.dram_tensor("x1tr", (D, n_rows), FP8, kind="ExternalInput").ap()
    x2tr = nc.dram_tensor("x2tr", (D, n_rows), FP8, kind="ExternalInput").ap()
    w1t = nc.dram_tensor("w1t", (D, D), FP8, kind="ExternalInput").ap()
    w2t = nc.dram_tensor("w2t", (D, D), FP8, kind="ExternalInput").ap()
    wflt = nc.dram_tensor("wflt", (D, D), BF16, kind="ExternalInput").ap()
    wfrt = nc.dram_tensor("wfrt", (D, D), BF16, kind="ExternalInput").ap()
    b1 = nc.dram_tensor("b1", (D,), F32, kind="ExternalInput").ap() if use_b1 else None
    b2 = nc.dram_tensor("b2", (D,), F32, kind="ExternalInput").ap() if use_b2 else None
    bfp = (
        nc.dram_tensor("bfp", (D,), F32, kind="ExternalInput").ap() if use_bf else None
    )
    out = nc.dram_tensor("out", (n_rows, D), F32, kind="ExternalOutput").ap()

    n_sb = n_rows // SB
    blocks_per_sb = SB // BLK

    with tile.TileContext(nc) as tc:
        from contextlib import ExitStack

        with ExitStack() as ctx:
            consts = ctx.enter_context(tc.tile_pool(name="consts", bufs=1))
            xf_pool = ctx.enter_context(tc.tile_pool(name="xf", bufs=2))
            xt_pool = ctx.enter_context(tc.tile_pool(name="xt", bufs=3))
            work = ctx.enter_context(tc.tile_pool(name="work", bufs=4))
            zpool = ctx.enter_context(tc.tile_pool(name="z", bufs=9))
            ztpool = ctx.enter_context(tc.tile_pool(name="zt", bufs=4))
            opool = ctx.enter_context(tc.tile_pool(name="o", bufs=3))
            stats = ctx.enter_context(tc.tile_pool(name="stats", bufs=6))
            psum_mm = ctx.enter_context(
                tc.tile_pool(name="psum_mm", bufs=2, space="PSUM")
            )
            psum_o = ctx.enter_context(
                tc.tile_pool(name="psum_o", bufs=1, space="PSUM")
            )
            psum_t = ctx.enter_context(
                tc.tile_pool(name="psum_t", bufs=2, space="PSUM")
            )

            # --- weights, chunk-split so the first matmuls unblock early ---
            w1t_sb = consts.tile([128, 8, D], FP8)
            w2t_sb = consts.tile([128, 8, D], FP8)
            wflt_sb = consts.tile([128, 8, D], BF16)
            wfrt_sb = consts.tile([128, 8, D], BF16)
            # strip-0 inputs, chunk-interleaved with the attention weights
            x1t0 = xt_pool.tile([128, 8, SB], FP8, name="x1t")
            x2t0 = xt_pool.tile([128, 8, SB], FP8, name="x2t")
            for c in range(8):
                nc.sync.dma_start(w1t_sb[:, c, :], w1t[ts(c, 128), :])
                nc.sync.dma_start(x2t0[:, c, :], x2tr[ts(c, 128), 0:SB])
            x1f0 = xf_pool.tile([128, blocks_per_sb, D], BF16, name="x1f")
            nc.sync.dma_start(x1f0, x1[0:SB, :].rearrange("(j p) d -> p j d", p=128))
            for c in range(8):
                nc.sync.dma_start(w2t_sb[:, c, :], w2t[ts(c, 128), :])
                nc.sync.dma_start(x1t0[:, c, :], x1tr[ts(c, 128), 0:SB])
            x2f0 = xf_pool.tile([128, blocks_per_sb, D], BF16, name="x2f")
            nc.sync.dma_start(x2f0, x2[0:SB, :].rearrange("(j p) d -> p j d", p=128))
            for c in range(8):
                nc.sync.dma_start(wflt_sb[:, c, :], wflt[ts(c, 128), :])
                nc.sync.dma_start(wfrt_sb[:, c, :], wfrt[ts(c, 128), :])
            ident = consts.tile([128, 128], BF16)
            make_identity(nc, ident)
            eps_sb = consts.tile([128, 1], F32)
            # a is computed at W_SCALE x true magnitude; LN is scale-invariant
            # so only eps needs the matching var scale.
            nc.vector.memset(eps_sb, EPS * W_SCALE * W_SCALE)
            b1_sb = b2_sb = bf_sb = None
            if use_b1:
                b1_sb = consts.tile([128, D], F32)
                nc.sync.dma_start(b1_sb, b1.partition_broadcast(128))
            if use_b2:
                b2_sb = consts.tile([128, D], F32)
                nc.sync.dma_start(b2_sb, b2.partition_broadcast(128))
            if use_bf:
                bf_sb = consts.tile([128, D], F32)
                nc.sync.dma_start(bf_sb, bfp.partition_broadcast(128))

            def emit_att(j, br, x1t, x2t):
                # attended = x_other @ W.T    [128 rows, 1024] (fp8 DoubleRow)
                xt_op = x2t if br == 0 else x1t
                wt = w1t_sb if br == 0 else w2t_sb
                ps = psum_mm.tile([128, D], F32, name="ps_attn")
                for t in range(4):
                    lhsT = xt_op[:, 2 * t : 2 * t + 2, ts(j, BLK)]
                    for h in range(2):
                        nc.tensor.matmul(
                            ps[:, ts(h, 512)],
                            lhsT,
                            wt[:, 2 * t : 2 * t + 2, ts(h, 512)],
                            start=(t == 0),
                            stop=(t == 3),
                            perf_mode=PM_DR,
                        )
                return ps

            def emit_ln(ps, j, br, x1f, x2f):
                # a = residual + attended (+ bias); z = (a - mean) * rstd
                res = x1f if br == 0 else x2f
                bias_sb = b1_sb if br == 0 else b2_sb
                # bf16 a: 2x DVE throughput on the whole LN chain; costs
                # <1e-4 rel err (LN output is scale-invariant, stats robust)
                a = work.tile([128, D], BF16, name="a")
                nc.vector.tensor_add(a, ps, res[:, j, :])
                if bias_sb is not None:
                    nc.vector.tensor_add(a, a, bias_sb)
                st = stats.tile([128, 2, 6], F32, name="st")
                nc.vector.bn_stats(st[:, 0, :], a[:, 0:512])
                nc.vector.bn_stats(st[:, 1, :], a[:, 512:1024])
                mv = stats.tile([128, 2], F32, name="mv")
                nc.vector.bn_aggr(mv, st)
                rstd = stats.tile([128, 1], F32, name="rstd")
                nc.scalar.activation(rstd, mv[:, 1:2], AF.Sqrt, bias=eps_sb)
                nc.vector.reciprocal(rstd, rstd)
                z = zpool.tile([128, D], BF16, name="z")
                nc.vector.tensor_scalar(
                    z, a, mv[:, 0:1], rstd, op0=ALU.subtract, op1=ALU.mult
                )
                return z

            def emit_transpose(z):
                pt = psum_t.tile([128, D], BF16, name="ptz", tag="pt")
                for c in range(8):
                    nc.tensor.transpose(pt[:, ts(c, 128)], z[:, ts(c, 128)], ident)
                zt = ztpool.tile([128, 8, BLK], BF16, name="zt")
                # psum->sbuf copy on the (otherwise idle) scalar engine
                nc.scalar.activation(zt, pt.rearrange("p (c n) -> p c n", c=8), AF.Copy)
                return zt

            def emit_fused(po, zt, br):
                wf = wflt_sb if br == 0 else wfrt_sb
                for c in range(8):
                    for h in range(2):
                        nc.tensor.matmul(
                            po[:, ts(h, 512)],
                            zt[:, c, :],
                            wf[:, c, ts(h, 512)],
                            start=(br == 0 and c == 0),
                            stop=(br == 1 and c == 7),
                        )

            def emit_out(po, r):
                o = opool.tile([128, D], F32, name="o")
                if bf_sb is not None:
                    nc.vector.tensor_add(o, po, bf_sb)
                else:
                    nc.scalar.activation(o, po, AF.Copy)
                nc.sync.dma_start(out[r : r + BLK, :], o)

            for s in range(n_sb):
                if s == 0:
                    x1t, x2t, x1f, x2f = x1t0, x2t0, x1f0, x2f0
                else:
                    sl = slice(s * SB, (s + 1) * SB)
                    x1t = xt_pool.tile([128, 8, SB], FP8, name="x1t")
                    nc.sync.dma_start(
                        x1t, x1tr[:, sl].rearrange("(c p) n -> p c n", p=128)
                    )
                    x2t = xt_pool.tile([128, 8, SB], FP8, name="x2t")
                    nc.sync.dma_start(
                        x2t, x2tr[:, sl].rearrange("(c p) n -> p c n", p=128)
                    )
                    x1f = xf_pool.tile([128, blocks_per_sb, D], BF16, name="x1f")
                    nc.sync.dma_start(
                        x1f, x1[sl, :].rearrange("(j p) d -> p j d", p=128)
                    )
                    x2f = xf_pool.tile([128, blocks_per_sb, D], BF16, name="x2f")
                    nc.sync.dma_start(
                        x2f, x2[sl, :].rearrange("(j p) d -> p j d", p=128)
                    )

                if s == 0 or s == n_sb - 1:
                    # Ramp strip: emit ALL attention groups first (they only
                    # need the early fp8 weights), so the PE queue never
                    # head-of-line blocks on the later bf16 fused weights.
                    # br-major order matches the DMA arrival order above.
                    # Same shape for the LAST strip: all LN chains complete
                    # while earlier fused groups run, so the tail drains as
                    # one dense fused stream.
                    orderA = [(j, br) for br in range(2) for j in range(blocks_per_sb)]
                    zs = {}
                    for j, br in orderA:
                        ps = emit_att(j, br, x1t, x2t)
                        zs[(j, br)] = emit_ln(ps, j, br, x1f, x2f)
                    orderB = [(j, br) for j in range(blocks_per_sb) for br in range(2)]
                    zts = {}
                    zts[orderB[0]] = emit_transpose(zs[orderB[0]])
                    zts[orderB[1]] = emit_transpose(zs[orderB[1]])
                    po = None
                    for k, (j, br) in enumerate(orderB):
                        if br == 0:
                            po = psum_o.tile([128, D], F32, name="po")
                        emit_fused(po, zts[(j, br)], br)
                        if k + 2 < len(orderB):
                            zts[orderB[k + 2]] = emit_transpose(zs[orderB[k + 2]])
                        if br == 1:
                            emit_out(po, s * SB + j * BLK)
                else:
                    for j in range(blocks_per_sb):
                        ps0 = emit_att(j, 0, x1t, x2t)
                        ps1 = emit_att(j, 1, x1t, x2t)
                        z0 = emit_ln(ps0, j, 0, x1f, x2f)
                        z1 = emit_ln(ps1, j, 1, x1f, x2f)
                        po = psum_o.tile([128, D], F32, name="po")
                        zt0 = emit_transpose(z0)
                        emit_fused(po, zt0, 0)
                        zt1 = emit_transpose(z1)
                        emit_fused(po, zt1, 1)
                        emit_out(po, s * SB + j * BLK)
    nc.compile()
    return nc


def _host_prep(Wv1, bv1, Wo1, bo1, Wv2, bv2, Wo2, bo2, g1, beta1, g2, beta2, Wf, bf):
    f32 = np.float32
    bfd = ml_dtypes.bfloat16
    Wv1, Wo1, Wv2, Wo2, Wf = (np.asarray(a, f32) for a in (Wv1, Wo1, Wv2, Wo2, Wf))
    bv1, bo1, bv2, bo2, bf = (np.asarray(a, f32) for a in (bv1, bo1, bv2, bo2, bf))
    g1, beta1, g2, beta2 = (np.asarray(a, f32) for a in (g1, beta1, g2, beta2))

    W1 = Wo1 @ Wv1  # [dout, din]
    W2 = Wo2 @ Wv2
    b1 = bv1 @ Wo1.T + bo1
    b2 = bv2 @ Wo2.T + bo2
    WfL = Wf[:, :D] * g1[None, :]
    WfR = Wf[:, D:] * g2[None, :]
    bfp = bf + beta1 @ Wf[:, :D].T + beta2 @ Wf[:, D:].T

    f8 = ml_dtypes.float8_e4m3fn
    weights = {
        "w1t": np.ascontiguousarray(W1.T * W_SCALE).astype(f8),
        "w2t": np.ascontiguousarray(W2.T * W_SCALE).astype(f8),
        "wflt": np.ascontiguousarray(WfL.T).astype(bfd),
        "wfrt": np.ascontiguousarray(WfR.T).astype(bfd),
    }
    use_b1 = bool(np.any(b1 != 0))
    use_b2 = bool(np.any(b2 != 0))
    use_bf = bool(np.any(bfp != 0))
    if use_b1:
        weights["b1"] = b1 * W_SCALE  # a is carried at W_SCALE x magnitude
    if use_b2:
        weights["b2"] = b2 * W_SCALE
    if use_bf:
        weights["bfp"] = bfp
    return weights, use_b1, use_b2, use_bf


def kernel(
    eeg_emb,
    ecg_emb,
    Wv1,
    bv1,
    Wo1,
    bo1,
    Wv2,
    bv2,
    Wo2,
    bo2,
    g1,
    beta1,
    g2,
    beta2,
    Wf,
    bf,
    _run_kwargs=None,
):
    from concourse.bass_utils import run_bass_kernel_spmd

    bfd = ml_dtypes.bfloat16
    f8 = ml_dtypes.float8_e4m3fn
    eeg32 = np.asarray(eeg_emb, np.float32)
    ecg32 = np.asarray(ecg_emb, np.float32)
    # residual path carried at W_SCALE x magnitude (matches fp8 attention psum)
    eeg = (eeg32 * W_SCALE).astype(bfd)
    ecg = (ecg32 * W_SCALE).astype(bfd)
    eegT = np.ascontiguousarray(eeg32.T).astype(f8)
    ecgT = np.ascontiguousarray(ecg32.T).astype(f8)
    weights, use_b1, use_b2, use_bf = _host_prep(
        Wv1, bv1, Wo1, bo1, Wv2, bv2, Wo2, bo2, g1, beta1, g2, beta2, Wf, bf
    )
    nc = build_program(ROWS_PER_CORE, use_b1, use_b2, use_bf)
    in_maps = []
    for i in range(N_CORES):
        sl = slice(i * ROWS_PER_CORE, (i + 1) * ROWS_PER_CORE)
        in_maps.append(
            {
                "x1": eeg[sl],
                "x2": ecg[sl],
                "x1tr": np.ascontiguousarray(eegT[:, sl]),
                "x2tr": np.ascontiguousarray(ecgT[:, sl]),
                **weights,
            }
        )
    res = run_bass_kernel_spmd(
        nc, in_maps, core_ids=list(range(N_CORES)), **(_run_kwargs or {})
    )
    out = np.concatenate([r["out"] for r in res.results], axis=0)
    if _run_kwargs:
        kernel.last_results = res
    return out


# revision 23
# speedup vs baseline: 1.2045x; 1.2045x over previous
"""Trainium2 Bass kernel for CrossAttentionFusion.

Math (kv seq_len == 1 collapses attention to two chained linear layers):
    eeg_att = ecg @ (Wo1 @ Wv1).T + (bv1 @ Wo1.T + bo1)
    eeg_out = LN(eeg + eeg_att) * g1 + beta1
    ecg_att = eeg @ (Wo2 @ Wv2).T + (bv2 @ Wo2.T + bo2)
    ecg_out = LN(ecg + ecg_att) * g2 + beta2
    out     = eeg_out @ WfL.T + ecg_out @ WfR.T + bf     (Wf = [WfL | WfR])

g/beta are folded into the fusion weights on the host:
    out = z1 @ (WfL*g1).T + z2 @ (WfR*g2).T + (bf + beta1@WfL.T + beta2@WfR.T)
where z = (a - mean(a)) * rsqrt(var(a) + eps) is the bare standardization.

The host supplies BOTH layouts of x (straight [rows, D] for the residual
path and transposed [D, rows] for the matmul lhsT), so the device does no
casting and no DMA/PE input transposes: the PE runs a dense back-to-back
matmul stream from the first microsecond.

The attention matmuls run in fp8-e4m3 with DoubleRow perf mode (2 fp8
MACs/cell/cycle): xT is quantized at scale 1, the folded attention weights
at scale 32. Because LN is scale-invariant, the 32x factor is folded into
the straight-x residual (host-scaled bf16) and eps (x1024) instead of being
divided out on-device. The fused matmul stays bf16 (fp8 there would breach
the error budget; measured 1.1e-2 vs 3.2e-2 rel err).

Per 128-row block:
  fp8 DoubleRow attention matmul (xT chunk stationary, W.T moving,
  rows-on-partition PSUM) -> residual + LN on DVE -> PE-transpose z ->
  bf16 fused matmul -> f32 store (psum->sbuf copy on the scalar engine).

Sharding: pure data parallel over the batch dim across 8 NeuronCores.
"""

import numpy as np
import ml_dtypes

import concourse.bass as bass
import concourse.mybir as mybir
import concourse.tile as tile
from concourse import bacc
from concourse.masks import make_identity

B, D = 32768, 1024
N_CORES = 8
ROWS_PER_CORE = B // N_CORES
EPS = 1e-5
W_SCALE = 32.0  # fp8 scale on attention weights; folded into residual + eps
F32 = mybir.dt.float32
BF16 = mybir.dt.bfloat16
FP8 = mybir.dt.float8e4
PM_DR = mybir.MatmulPerfMode.DoubleRow
BLK = 128  # row block (psum partition tile)
SB = 512  # super-block rows per strip
ts = bass.ts
AF = mybir.ActivationFunctionType
ALU = mybir.AluOpType


def build_program(n_rows=ROWS_PER_CORE, use_b1=False, use_b2=False, use_bf=False):
    nc = bacc.Bacc("TRN2", target_bir_lowering=False, debug=False)
    x1 = nc.dram_tensor("x1", (n_rows, D), BF16, kind="ExternalInput").ap()
    x2 = nc.dram_tensor("x2", (n_rows, D), BF16, kind="ExternalInput").ap()
    x1tr = nc.dram_tensor("x1tr", (D, n_rows), FP8, kind="ExternalInput").ap()
    x2tr = nc.dram_tensor("x2tr", (D, n_rows), FP8, kind="ExternalInput").ap()
    w1t = nc.dram_tensor("w1t", (D, D), FP8, kind="ExternalInput").ap()
    w2t = nc.dram_tensor("w2t", (D, D), FP8, kind="ExternalInput").ap()
    wflt = nc.dram_tensor("wflt", (D, D), BF16, kind="ExternalInput").ap()
    wfrt = nc.dram_tensor("wfrt", (D, D), BF16, kind="ExternalInput").ap()
    b1 = nc.dram_tensor("b1", (D,), F32, kind="ExternalInput").ap() if use_b1 else None
    b2 = nc.dram_tensor("b2", (D,), F32, kind="ExternalInput").ap() if use_b2 else None
    bfp = (
        nc.dram_tensor("bfp", (D,), F32, kind="ExternalInput").ap() if use_bf else None
    )
    out = nc.dram_tensor("out", (n_rows, D), F32, kind="ExternalOutput").ap()

    n_sb = n_rows // SB
    blocks_per_sb = SB // BLK

    with tile.TileContext(nc) as tc:
        from contextlib import ExitStack

        with ExitStack() as ctx:
            consts = ctx.enter_context(tc.tile_pool(name="consts", bufs=1))
            xf_pool = ctx.enter_context(tc.tile_pool(name="xf", bufs=2))
            xt_pool = ctx.enter_context(tc.tile_pool(name="xt", bufs=3))
            work = ctx.enter_context(tc.tile_pool(name="work", bufs=4))
            zpool = ctx.enter_context(tc.tile_pool(name="z", bufs=9))
            ztpool = ctx.enter_context(tc.tile_pool(name="zt", bufs=4))
            opool = ctx.enter_context(tc.tile_pool(name="o", bufs=3))
            stats = ctx.enter_context(tc.tile_pool(name="stats", bufs=6))
            psum_mm = ctx.enter_context(
                tc.tile_pool(name="psum_mm", bufs=2, space="PSUM")
            )
            psum_o = ctx.enter_context(
                tc.tile_pool(name="psum_o", bufs=1, space="PSUM")
            )
            psum_t = ctx.enter_context(
                tc.tile_pool(name="psum_t", bufs=2, space="PSUM")
            )

            # --- weights, chunk-split so the first matmuls unblock early ---
            w1t_sb = consts.tile([128, 8, D], FP8)
            w2t_sb = consts.tile([128, 8, D], FP8)
            wflt_sb = consts.tile([128, 8, D], BF16)
            wfrt_sb = consts.tile([128, 8, D], BF16)
            # strip-0 inputs, chunk-interleaved with the attention weights
            x1t0 = xt_pool.tile([128, 8, SB], FP8, name="x1t")
            x2t0 = xt_pool.tile([128, 8, SB], FP8, name="x2t")
            for c in range(8):
                nc.sync.dma_start(w1t_sb[:, c, :], w1t[ts(c, 128), :])
                nc.sync.dma_start(x2t0[:, c, :], x2tr[ts(c, 128), 0:SB])
            x1f0 = xf_pool.tile([128, blocks_per_sb, D], BF16, name="x1f")
            nc.sync.dma_start(x1f0, x1[0:SB, :].rearrange("(j p) d -> p j d", p=128))
            for c in range(8):
                nc.sync.dma_start(w2t_sb[:, c, :], w2t[ts(c, 128), :])
                nc.sync.dma_start(x1t0[:, c, :], x1tr[ts(c, 128), 0:SB])
            x2f0 = xf_pool.tile([128, blocks_per_sb, D], BF16, name="x2f")
            nc.sync.dma_start(x2f0, x2[0:SB, :].rearrange("(j p) d -> p j d", p=128))
            for c in range(8):
                nc.sync.dma_start(wflt_sb[:, c, :], wflt[ts(c, 128), :])
                nc.sync.dma_start(wfrt_sb[:, c, :], wfrt[ts(c, 128), :])
            ident = consts.tile([128, 128], BF16)
            make_identity(nc, ident)
            eps_sb = consts.tile([128, 1], F32)
            # a is computed at W_SCALE x true magnitude; LN is scale-invariant
            # so only eps needs the matching var scale.
            nc.vector.memset(eps_sb, EPS * W_SCALE * W_SCALE)
            b1_sb = b2_sb = bf_sb = None
            if use_b1:
                b1_sb = consts.tile([128, D], F32)
                nc.sync.dma_start(b1_sb, b1.partition_broadcast(128))
            if use_b2:
                b2_sb = consts.tile([128, D], F32)
                nc.sync.dma_start(b2_sb, b2.partition_broadcast(128))
            if use_bf:
                bf_sb = consts.tile([128, D], F32)
                nc.sync.dma_start(bf_sb, bfp.partition_broadcast(128))

            def emit_att(j, br, x1t, x2t):
                # attended = x_other @ W.T    [128 rows, 1024] (fp8 DoubleRow)
                xt_op = x2t if br == 0 else x1t
                wt = w1t_sb if br == 0 else w2t_sb
                ps = psum_mm.tile([128, D], F32, name="ps_attn")
                for t in range(4):
                    lhsT = xt_op[:, 2 * t : 2 * t + 2, ts(j, BLK)]
                    for h in range(2):
                        nc.tensor.matmul(
                            ps[:, ts(h, 512)],
                            lhsT,
                            wt[:, 2 * t : 2 * t + 2, ts(h, 512)],
                            start=(t == 0),
                            stop=(t == 3),
                            perf_mode=PM_DR,
                        )
                return ps

            def emit_ln(ps, j, br, x1f, x2f):
                # a = residual + attended (+ bias); z = (a - mean) * rstd
                res = x1f if br == 0 else x2f
                bias_sb = b1_sb if br == 0 else b2_sb
                # NB: bf16 `a` measured SLOWER overall — DVE 2-port mode for
                # 16-bit ops contends for SBUF ports with the PE rhs stream
                # (MM avg 344->413ns). Keep f32.
                a = work.tile([128, D], F32, name="a")
                nc.vector.tensor_add(a, ps, res[:, j, :])
                if bias_sb is not None:
                    nc.vector.tensor_add(a, a, bias_sb)
                st = stats.tile([128, 2, 6], F32, name="st")
                nc.vector.bn_stats(st[:, 0, :], a[:, 0:512])
                nc.vector.bn_stats(st[:, 1, :], a[:, 512:1024])
                mv = stats.tile([128, 2], F32, name="mv")
                nc.vector.bn_aggr(mv, st)
                rstd = stats.tile([128, 1], F32, name="rstd")
                nc.scalar.activation(rstd, mv[:, 1:2], AF.Sqrt, bias=eps_sb)
                nc.vector.reciprocal(rstd, rstd)
                z = zpool.tile([128, D], BF16, name="z")
                nc.vector.tensor_scalar(
                    z, a, mv[:, 0:1], rstd, op0=ALU.subtract, op1=ALU.mult
                )
                return z

            def emit_transpose(z):
                pt = psum_t.tile([128, D], BF16, name="ptz", tag="pt")
                for c in range(8):
                    nc.tensor.transpose(pt[:, ts(c, 128)], z[:, ts(c, 128)], ident)
                zt = ztpool.tile([128, 8, BLK], BF16, name="zt")
                # psum->sbuf copy on the (otherwise idle) scalar engine
                nc.scalar.activation(zt, pt.rearrange("p (c n) -> p c n", c=8), AF.Copy)
                return zt

            def emit_fused(po, zt, br):
                wf = wflt_sb if br == 0 else wfrt_sb
                for c in range(8):
                    for h in range(2):
                        nc.tensor.matmul(
                            po[:, ts(h, 512)],
                            zt[:, c, :],
                            wf[:, c, ts(h, 512)],
                            start=(br == 0 and c == 0),
                            stop=(br == 1 and c == 7),
                        )

            def emit_out(po, r):
                o = opool.tile([128, D], F32, name="o")
                if bf_sb is not None:
                    nc.vector.tensor_add(o, po, bf_sb)
                else:
                    nc.scalar.activation(o, po, AF.Copy)
                nc.sync.dma_start(out[r : r + BLK, :], o)

            for s in range(n_sb):
                if s == 0:
                    x1t, x2t, x1f, x2f = x1t0, x2t0, x1f0, x2f0
                else:
                    sl = slice(s * SB, (s + 1) * SB)
                    x1t = xt_pool.tile([128, 8, SB], FP8, name="x1t")
                    nc.sync.dma_start(
                        x1t, x1tr[:, sl].rearrange("(c p) n -> p c n", p=128)
                    )
                    x2t = xt_pool.tile([128, 8, SB], FP8, name="x2t")
                    nc.sync.dma_start(
                        x2t, x2tr[:, sl].rearrange("(c p) n -> p c n", p=128)
                    )
                    x1f = xf_pool.tile([128, blocks_per_sb, D], BF16, name="x1f")
                    nc.sync.dma_start(
                        x1f, x1[sl, :].rearrange("(j p) d -> p j d", p=128)
                    )
                    x2f = xf_pool.tile([128, blocks_per_sb, D], BF16, name="x2f")
                    nc.sync.dma_start(
                        x2f, x2[sl, :].rearrange("(j p) d -> p j d", p=128)
                    )

                if s == 0 or s == n_sb - 1:
                    # Ramp strip: emit ALL attention groups first (they only
                    # need the early fp8 weights), so the PE queue never
                    # head-of-line blocks on the later bf16 fused weights.
                    # br-major order matches the DMA arrival order above.
                    # Same shape for the LAST strip: all LN chains complete
                    # while earlier fused groups run, so the tail drains as
                    # one dense fused stream.
                    orderA = [(j, br) for br in range(2) for j in range(blocks_per_sb)]
                    zs = {}
                    for j, br in orderA:
                        ps = emit_att(j, br, x1t, x2t)
                        zs[(j, br)] = emit_ln(ps, j, br, x1f, x2f)
                    orderB = [(j, br) for j in range(blocks_per_sb) for br in range(2)]
                    zts = {}
                    zts[orderB[0]] = emit_transpose(zs[orderB[0]])
                    zts[orderB[1]] = emit_transpose(zs[orderB[1]])
                    po = None
                    for k, (j, br) in enumerate(orderB):
                        if br == 0:
                            po = psum_o.tile([128, D], F32, name="po")
                        emit_fused(po, zts[(j, br)], br)
                        if k + 2 < len(orderB):
                            zts[orderB[k + 2]] = emit_transpose(zs[orderB[k + 2]])
                        if br == 1:
                            emit_out(po, s * SB + j * BLK)
                else:
                    for j in range(blocks_per_sb):
                        ps0 = emit_att(j, 0, x1t, x2t)
                        ps1 = emit_att(j, 1, x1t, x2t)
                        z0 = emit_ln(ps0, j, 0, x1f, x2f)
                        z1 = emit_ln(ps1, j, 1, x1f, x2f)
                        po = psum_o.tile([128, D], F32, name="po")
                        zt0 = emit_transpose(z0)
                        emit_fused(po, zt0, 0)
                        zt1 = emit_transpose(z1)
                        emit_fused(po, zt1, 1)
                        emit_out(po, s * SB + j * BLK)
    nc.compile()
    return nc


def _host_prep(Wv1, bv1, Wo1, bo1, Wv2, bv2, Wo2, bo2, g1, beta1, g2, beta2, Wf, bf):
    f32 = np.float32
    bfd = ml_dtypes.bfloat16
    Wv1, Wo1, Wv2, Wo2, Wf = (np.asarray(a, f32) for a in (Wv1, Wo1, Wv2, Wo2, Wf))
    bv1, bo1, bv2, bo2, bf = (np.asarray(a, f32) for a in (bv1, bo1, bv2, bo2, bf))
    g1, beta1, g2, beta2 = (np.asarray(a, f32) for a in (g1, beta1, g2, beta2))

    W1 = Wo1 @ Wv1  # [dout, din]
    W2 = Wo2 @ Wv2
    b1 = bv1 @ Wo1.T + bo1
    b2 = bv2 @ Wo2.T + bo2
    WfL = Wf[:, :D] * g1[None, :]
    WfR = Wf[:, D:] * g2[None, :]
    bfp = bf + beta1 @ Wf[:, :D].T + beta2 @ Wf[:, D:].T

    f8 = ml_dtypes.float8_e4m3fn
    weights = {
        "w1t": np.ascontiguousarray(W1.T * W_SCALE).astype(f8),
        "w2t": np.ascontiguousarray(W2.T * W_SCALE).astype(f8),
        "wflt": np.ascontiguousarray(WfL.T).astype(bfd),
        "wfrt": np.ascontiguousarray(WfR.T).astype(bfd),
    }
    use_b1 = bool(np.any(b1 != 0))
    use_b2 = bool(np.any(b2 != 0))
    use_bf = bool(np.any(bfp != 0))
    if use_b1:
        weights["b1"] = b1 * W_SCALE  # a is carried at W_SCALE x magnitude
    if use_b2:
        weights["b2"] = b2 * W_SCALE
    if use_bf:
        weights["bfp"] = bfp
    return weights, use_b1, use_b2, use_bf


def kernel(
    eeg_emb,
    ecg_emb,
    Wv1,
    bv1,
    Wo1,
    bo1,
    Wv2,
    bv2,
    Wo2,
    bo2,
    g1,
    beta1,
    g2,
    beta2,
    Wf,
    bf,
    _run_kwargs=None,
):
    from concourse.bass_utils import run_bass_kernel_spmd

    bfd = ml_dtypes.bfloat16
    f8 = ml_dtypes.float8_e4m3fn
    eeg32 = np.asarray(eeg_emb, np.float32)
    ecg32 = np.asarray(ecg_emb, np.float32)
    # residual path carried at W_SCALE x magnitude (matches fp8 attention psum)
    eeg = (eeg32 * W_SCALE).astype(bfd)
    ecg = (ecg32 * W_SCALE).astype(bfd)
    eegT = np.ascontiguousarray(eeg32.T).astype(f8)
    ecgT = np.ascontiguousarray(ecg32.T).astype(f8)
    weights, use_b1, use_b2, use_bf = _host_prep(
        Wv1, bv1, Wo1, bo1, Wv2, bv2, Wo2, bo2, g1, beta1, g2, beta2, Wf, bf
    )
    nc = build_program(ROWS_PER_CORE, use_b1, use_b2, use_bf)
    in_maps = []
    for i in range(N_CORES):
        sl = slice(i * ROWS_PER_CORE, (i + 1) * ROWS_PER_CORE)
        in_maps.append(
            {
                "x1": eeg[sl],
                "x2": ecg[sl],
                "x1tr": np.ascontiguousarray(eegT[:, sl]),
                "x2tr": np.ascontiguousarray(ecgT[:, sl]),
                **weights,
            }
        )
    res = run_bass_kernel_spmd(
        nc, in_maps, core_ids=list(range(N_CORES)), **(_run_kwargs or {})
    )
    out = np.concatenate([r["out"] for r in res.results], axis=0)
    if _run_kwargs:
        kernel.last_results = res
    return out


# revision 25
# speedup vs baseline: 1.2074x; 1.0024x over previous
"""Trainium2 Bass kernel for CrossAttentionFusion.

Math (kv seq_len == 1 collapses attention to two chained linear layers):
    eeg_att = ecg @ (Wo1 @ Wv1).T + (bv1 @ Wo1.T + bo1)
    eeg_out = LN(eeg + eeg_att) * g1 + beta1
    ecg_att = eeg @ (Wo2 @ Wv2).T + (bv2 @ Wo2.T + bo2)
    ecg_out = LN(ecg + ecg_att) * g2 + beta2
    out     = eeg_out @ WfL.T + ecg_out @ WfR.T + bf     (Wf = [WfL | WfR])

g/beta are folded into the fusion weights on the host:
    out = z1 @ (WfL*g1).T + z2 @ (WfR*g2).T + (bf + beta1@WfL.T + beta2@WfR.T)
where z = (a - mean(a)) * rsqrt(var(a) + eps) is the bare standardization.

The host supplies BOTH layouts of x (straight [rows, D] for the residual
path and transposed [D, rows] for the matmul lhsT), so the device does no
casting and no DMA/PE input transposes: the PE runs a dense back-to-back
matmul stream from the first microsecond.

The attention matmuls run in fp8-e4m3 with DoubleRow perf mode (2 fp8
MACs/cell/cycle): xT is quantized at scale 1, the folded attention weights
at scale 32. Because LN is scale-invariant, the 32x factor is folded into
the straight-x residual (host-scaled bf16) and eps (x1024) instead of being
divided out on-device. The fused matmul stays bf16 (fp8 there would breach
the error budget; measured 1.1e-2 vs 3.2e-2 rel err).

Per 128-row block:
  fp8 DoubleRow attention matmul (xT chunk stationary, W.T moving,
  rows-on-partition PSUM) -> residual + LN on DVE -> PE-transpose z ->
  bf16 fused matmul -> f32 store (psum->sbuf copy on the scalar engine).

Sharding: pure data parallel over the batch dim across 8 NeuronCores.
"""

import numpy as np
import ml_dtypes

import concourse.bass as bass
import concourse.mybir as mybir
import concourse.tile as tile
from concourse import bacc
from concourse.masks import make_identity

B, D = 32768, 1024
N_CORES = 8
ROWS_PER_CORE = B // N_CORES
EPS = 1e-5
W_SCALE = 32.0  # fp8 scale on attention weights; folded into residual + eps
F32 = mybir.dt.float32
BF16 = mybir.dt.bfloat16
FP8 = mybir.dt.float8e4
PM_DR = mybir.MatmulPerfMode.DoubleRow
BLK = 128  # row block (psum partition tile)
SB = 512  # super-block rows per strip
ts = bass.ts
AF = mybir.ActivationFunctionType
ALU = mybir.AluOpType


def build_program(n_rows=ROWS_PER_CORE, use_b1=False, use_b2=False, use_bf=False):
    nc = bacc.Bacc("TRN2", target_bir_lowering=False, debug=False)
    x1 = nc.dram_tensor("x1", (n_rows, D), BF16, kind="ExternalInput").ap()
    x2 = nc.dram_tensor("x2", (n_rows, D), BF16, kind="ExternalInput").ap()
    x1tr = nc.dram_tensor("x1tr", (D, n_rows), FP8, kind="ExternalInput").ap()
    x2tr = nc.dram_tensor("x2tr", (D, n_rows), FP8, kind="ExternalInput").ap()
    w1t = nc.dram_tensor("w1t", (D, D), FP8, kind="ExternalInput").ap()
    w2t = nc.dram_tensor("w2t", (D, D), FP8, kind="ExternalInput").ap()
    wflt = nc.dram_tensor("wflt", (D, D), BF16, kind="ExternalInput").ap()
    wfrt = nc.dram_tensor("wfrt", (D, D), BF16, kind="ExternalInput").ap()
    b1 = nc.dram_tensor("b1", (D,), F32, kind="ExternalInput").ap() if use_b1 else None
    b2 = nc.dram_tensor("b2", (D,), F32, kind="ExternalInput").ap() if use_b2 else None
    bfp = (
        nc.dram_tensor("bfp", (D,), F32, kind="ExternalInput").ap() if use_bf else None
    )
    out = nc.dram_tensor("out", (n_rows, D), F32, kind="ExternalOutput").ap()

    n_sb = n_rows // SB
    blocks_per_sb = SB // BLK

    with tile.TileContext(nc) as tc:
        from contextlib import ExitStack

        with ExitStack() as ctx:
            consts = ctx.enter_context(tc.tile_pool(name="consts", bufs=1))
            xf_pool = ctx.enter_context(tc.tile_pool(name="xf", bufs=2))
            xt_pool = ctx.enter_context(tc.tile_pool(name="xt", bufs=3))
            work = ctx.enter_context(tc.tile_pool(name="work", bufs=4))
            zpool = ctx.enter_context(tc.tile_pool(name="z", bufs=9))
            ztpool = ctx.enter_context(tc.tile_pool(name="zt", bufs=4))
            opool = ctx.enter_context(tc.tile_pool(name="o", bufs=3))
            stats = ctx.enter_context(tc.tile_pool(name="stats", bufs=6))
            psum_mm = ctx.enter_context(
                tc.tile_pool(name="psum_mm", bufs=2, space="PSUM")
            )
            psum_o = ctx.enter_context(
                tc.tile_pool(name="psum_o", bufs=1, space="PSUM")
            )
            psum_t = ctx.enter_context(
                tc.tile_pool(name="psum_t", bufs=2, space="PSUM")
            )

            # --- weights, chunk-split so the first matmuls unblock early ---
            w1t_sb = consts.tile([128, 8, D], FP8)
            w2t_sb = consts.tile([128, 8, D], FP8)
            wflt_sb = consts.tile([128, 8, D], BF16)
            wfrt_sb = consts.tile([128, 8, D], BF16)
            # strip-0 inputs, chunk-interleaved with the attention weights
            x1t0 = xt_pool.tile([128, 8, SB], FP8, name="x1t")
            x2t0 = xt_pool.tile([128, 8, SB], FP8, name="x2t")
            # strip-0 residuals as per-block tiles loaded early, so the LN
            # chains (which gate attention-psum recycling) unlock per block
            # as soon as 256 KB lands instead of after the whole 1 MB strip.
            x1f0 = xf_pool.tile([128, blocks_per_sb, D], BF16, name="x1f")
            x2f0 = xf_pool.tile([128, blocks_per_sb, D], BF16, name="x2f")
            nc.sync.dma_start(x1f0[:, 0, :], x1[0:BLK, :])
            for c in range(8):
                nc.sync.dma_start(w1t_sb[:, c, :], w1t[ts(c, 128), :])
                nc.sync.dma_start(x2t0[:, c, :], x2tr[ts(c, 128), 0:SB])
                if c < 3:
                    j = c + 1
                    nc.sync.dma_start(
                        x1f0[:, j, :], x1[j * BLK : (j + 1) * BLK, :]
                    )
            nc.sync.dma_start(x2f0[:, 0, :], x2[0:BLK, :])
            for c in range(8):
                nc.sync.dma_start(w2t_sb[:, c, :], w2t[ts(c, 128), :])
                nc.sync.dma_start(x1t0[:, c, :], x1tr[ts(c, 128), 0:SB])
                if c < 3:
                    j = c + 1
                    nc.sync.dma_start(
                        x2f0[:, j, :], x2[j * BLK : (j + 1) * BLK, :]
                    )
            for c in range(8):
                nc.sync.dma_start(wflt_sb[:, c, :], wflt[ts(c, 128), :])
                nc.sync.dma_start(wfrt_sb[:, c, :], wfrt[ts(c, 128), :])
            ident = consts.tile([128, 128], BF16)
            make_identity(nc, ident)
            eps_sb = consts.tile([128, 1], F32)
            # a is computed at W_SCALE x true magnitude; LN is scale-invariant
            # so only eps needs the matching var scale.
            nc.vector.memset(eps_sb, EPS * W_SCALE * W_SCALE)
            b1_sb = b2_sb = bf_sb = None
            if use_b1:
                b1_sb = consts.tile([128, D], F32)
                nc.sync.dma_start(b1_sb, b1.partition_broadcast(128))
            if use_b2:
                b2_sb = consts.tile([128, D], F32)
                nc.sync.dma_start(b2_sb, b2.partition_broadcast(128))
            if use_bf:
                bf_sb = consts.tile([128, D], F32)
                nc.sync.dma_start(bf_sb, bfp.partition_broadcast(128))

            def emit_att(j, br, x1t, x2t):
                # attended = x_other @ W.T    [128 rows, 1024] (fp8 DoubleRow)
                xt_op = x2t if br == 0 else x1t
                wt = w1t_sb if br == 0 else w2t_sb
                ps = psum_mm.tile([128, D], F32, name="ps_attn")
                for t in range(4):
                    lhsT = xt_op[:, 2 * t : 2 * t + 2, ts(j, BLK)]
                    for h in range(2):
                        nc.tensor.matmul(
                            ps[:, ts(h, 512)],
                            lhsT,
                            wt[:, 2 * t : 2 * t + 2, ts(h, 512)],
                            start=(t == 0),
                            stop=(t == 3),
                            perf_mode=PM_DR,
                        )
                return ps

            def emit_ln(ps, j, br, x1f, x2f):
                # a = residual + attended (+ bias); z = (a - mean) * rstd
                res = x1f if br == 0 else x2f
                bias_sb = b1_sb if br == 0 else b2_sb
                # NB: bf16 `a` measured SLOWER overall — DVE 2-port mode for
                # 16-bit ops contends for SBUF ports with the PE rhs stream
                # (MM avg 344->413ns). Keep f32.
                a = work.tile([128, D], F32, name="a")
                nc.vector.tensor_add(a, ps, res[:, j, :])
                if bias_sb is not None:
                    nc.vector.tensor_add(a, a, bias_sb)
                st = stats.tile([128, 2, 6], F32, name="st")
                nc.vector.bn_stats(st[:, 0, :], a[:, 0:512])
                nc.vector.bn_stats(st[:, 1, :], a[:, 512:1024])
                mv = stats.tile([128, 2], F32, name="mv")
                nc.vector.bn_aggr(mv, st)
                rstd = stats.tile([128, 1], F32, name="rstd")
                nc.scalar.activation(rstd, mv[:, 1:2], AF.Sqrt, bias=eps_sb)
                nc.vector.reciprocal(rstd, rstd)
                z = zpool.tile([128, D], BF16, name="z")
                nc.vector.tensor_scalar(
                    z, a, mv[:, 0:1], rstd, op0=ALU.subtract, op1=ALU.mult
                )
                return z

            def emit_transpose(z):
                pt = psum_t.tile([128, D], BF16, name="ptz", tag="pt")
                for c in range(8):
                    nc.tensor.transpose(pt[:, ts(c, 128)], z[:, ts(c, 128)], ident)
                zt = ztpool.tile([128, 8, BLK], BF16, name="zt")
                # psum->sbuf copy on the (otherwise idle) scalar engine
                nc.scalar.activation(zt, pt.rearrange("p (c n) -> p c n", c=8), AF.Copy)
                return zt

            def emit_fused(po, zt, br):
                wf = wflt_sb if br == 0 else wfrt_sb
                for c in range(8):
                    for h in range(2):
                        nc.tensor.matmul(
                            po[:, ts(h, 512)],
                            zt[:, c, :],
                            wf[:, c, ts(h, 512)],
                            start=(br == 0 and c == 0),
                            stop=(br == 1 and c == 7),
                        )

            def emit_out(po, r):
                o = opool.tile([128, D], F32, name="o")
                if bf_sb is not None:
                    nc.vector.tensor_add(o, po, bf_sb)
                else:
                    nc.scalar.activation(o, po, AF.Copy)
                nc.sync.dma_start(out[r : r + BLK, :], o)

            for s in range(n_sb):
                if s == 0:
                    x1t, x2t, x1f, x2f = x1t0, x2t0, x1f0, x2f0
                else:
                    sl = slice(s * SB, (s + 1) * SB)
                    x1t = xt_pool.tile([128, 8, SB], FP8, name="x1t")
                    nc.sync.dma_start(
                        x1t, x1tr[:, sl].rearrange("(c p) n -> p c n", p=128)
                    )
                    x2t = xt_pool.tile([128, 8, SB], FP8, name="x2t")
                    nc.sync.dma_start(
                        x2t, x2tr[:, sl].rearrange("(c p) n -> p c n", p=128)
                    )
                    x1f = xf_pool.tile([128, blocks_per_sb, D], BF16, name="x1f")
                    nc.sync.dma_start(
                        x1f, x1[sl, :].rearrange("(j p) d -> p j d", p=128)
                    )
                    x2f = xf_pool.tile([128, blocks_per_sb, D], BF16, name="x2f")
                    nc.sync.dma_start(
                        x2f, x2[sl, :].rearrange("(j p) d -> p j d", p=128)
                    )

                if s == 0 or s == n_sb - 1:
                    # Ramp strip: emit ALL attention groups first (they only
                    # need the early fp8 weights), so the PE queue never
                    # head-of-line blocks on the later bf16 fused weights.
                    # br-major order matches the DMA arrival order above.
                    # Same shape for the LAST strip: all LN chains complete
                    # while earlier fused groups run, so the tail drains as
                    # one dense fused stream.
                    orderA = [(j, br) for br in range(2) for j in range(blocks_per_sb)]
                    zs = {}
                    for j, br in orderA:
                        ps = emit_att(j, br, x1t, x2t)
                        zs[(j, br)] = emit_ln(ps, j, br, x1f, x2f)
                    orderB = [(j, br) for j in range(blocks_per_sb) for br in range(2)]
                    zts = {}
                    zts[orderB[0]] = emit_transpose(zs[orderB[0]])
                    zts[orderB[1]] = emit_transpose(zs[orderB[1]])
                    po = None
                    for k, (j, br) in enumerate(orderB):
                        if br == 0:
                            po = psum_o.tile([128, D], F32, name="po")
                        emit_fused(po, zts[(j, br)], br)
                        if k + 2 < len(orderB):
                            zts[orderB[k + 2]] = emit_transpose(zs[orderB[k + 2]])
                        if br == 1:
                            emit_out(po, s * SB + j * BLK)
                else:
                    for j in range(blocks_per_sb):
                        ps0 = emit_att(j, 0, x1t, x2t)
                        ps1 = emit_att(j, 1, x1t, x2t)
                        z0 = emit_ln(ps0, j, 0, x1f, x2f)
                        z1 = emit_ln(ps1, j, 1, x1f, x2f)
                        po = psum_o.tile([128, D], F32, name="po")
                        zt0 = emit_transpose(z0)
                        emit_fused(po, zt0, 0)
                        zt1 = emit_transpose(z1)
                        emit_fused(po, zt1, 1)
                        emit_out(po, s * SB + j * BLK)
    nc.compile()
    return nc


def _host_prep(Wv1, bv1, Wo1, bo1, Wv2, bv2, Wo2, bo2, g1, beta1, g2, beta2, Wf, bf):
    f32 = np.float32
    bfd = ml_dtypes.bfloat16
    Wv1, Wo1, Wv2, Wo2, Wf = (np.asarray(a, f32) for a in (Wv1, Wo1, Wv2, Wo2, Wf))
    bv1, bo1, bv2, bo2, bf = (np.asarray(a, f32) for a in (bv1, bo1, bv2, bo2, bf))
    g1, beta1, g2, beta2 = (np.asarray(a, f32) for a in (g1, beta1, g2, beta2))

    W1 = Wo1 @ Wv1  # [dout, din]
    W2 = Wo2 @ Wv2
    b1 = bv1 @ Wo1.T + bo1
    b2 = bv2 @ Wo2.T + bo2
    WfL = Wf[:, :D] * g1[None, :]
    WfR = Wf[:, D:] * g2[None, :]
    bfp = bf + beta1 @ Wf[:, :D].T + beta2 @ Wf[:, D:].T

    f8 = ml_dtypes.float8_e4m3fn
    weights = {
        "w1t": np.ascontiguousarray(W1.T * W_SCALE).astype(f8),
        "w2t": np.ascontiguousarray(W2.T * W_SCALE).astype(f8),
        "wflt": np.ascontiguousarray(WfL.T).astype(bfd),
        "wfrt": np.ascontiguousarray(WfR.T).astype(bfd),
    }
    use_b1 = bool(np.any(b1 != 0))
    use_b2 = bool(np.any(b2 != 0))
    use_bf = bool(np.any(bfp != 0))
    if use_b1:
        weights["b1"] = b1 * W_SCALE  # a is carried at W_SCALE x magnitude
    if use_b2:
        weights["b2"] = b2 * W_SCALE
    if use_bf:
        weights["bfp"] = bfp
    return weights, use_b1, use_b2, use_bf


def kernel(
    eeg_emb,
    ecg_emb,
    Wv1,
    bv1,
    Wo1,
    bo1,
    Wv2,
    bv2,
    Wo2,
    bo2,
    g1,
    beta1,
    g2,
    beta2,
    Wf,
    bf,
    _run_kwargs=None,
):
    from concourse.bass_utils import run_bass_kernel_spmd

    bfd = ml_dtypes.bfloat16
    f8 = ml_dtypes.float8_e4m3fn
    eeg32 = np.asarray(eeg_emb, np.float32)
    ecg32 = np.asarray(ecg_emb, np.float32)
    # residual path carried at W_SCALE x magnitude (matches fp8 attention psum)
    eeg = (eeg32 * W_SCALE).astype(bfd)
    ecg = (ecg32 * W_SCALE).astype(bfd)
    eegT = np.ascontiguousarray(eeg32.T).astype(f8)
    ecgT = np.ascontiguousarray(ecg32.T).astype(f8)
    weights, use_b1, use_b2, use_bf = _host_prep(
        Wv1, bv1, Wo1, bo1, Wv2, bv2, Wo2, bo2, g1, beta1, g2, beta2, Wf, bf
    )
    nc = build_program(ROWS_PER_CORE, use_b1, use_b2, use_bf)
    in_maps = []
    for i in range(N_CORES):
        sl = slice(i * ROWS_PER_CORE, (i + 1) * ROWS_PER_CORE)
        in_maps.append(
            {
                "x1": eeg[sl],
                "x2": ecg[sl],
                "x1tr": np.ascontiguousarray(eegT[:, sl]),
                "x2tr": np.ascontiguousarray(ecgT[:, sl]),
                **weights,
            }
        )
    res = run_bass_kernel_spmd(
        nc, in_maps, core_ids=list(range(N_CORES)), **(_run_kwargs or {})
    )
    out = np.concatenate([r["out"] for r in res.results], axis=0)
    if _run_kwargs:
        kernel.last_results = res
    return out
